# revision 19
# baseline (speedup 1.0000x reference)
"""2-layer GCN on 8 Trainium2 NeuronCores (Bass/Tile, SPMD).

softmax(A @ relu(A @ (X@W1) + b1) @ W2 + b2), N=50k nodes, E=800k edges.

Strategy (1D graph partition, token-packed gathers, 4 SWDGE queues):
- Nodes sharded 6250/core by global in-degree rank round-robin (rank k ->
  core k%8, local id k//8), padded to 6272 = 49*128 table rows per core.
  Local ids are descending-degree, so 128-node dst tiles have near-uniform
  degree and fixed per-tile slot counts waste little padding.
- Edges partitioned by dst owner. Tables are token-packed into 256B gather
  elements (fp32 throughout -- softmax needs logit errors < 1e-2): layer-1
  XW1 [50176, 64] -> 25088 pair-tokens of 512B; layer-2 HW2 [50176, 16] ->
  12544 4-row tokens of 256B. Token counts fit int16 gather indices, so no
  index-class split is needed.
- Per dst node, in-edges are deduped by (dst, token) into slots; each slot
  gathers one 256B token and a per-sub-row weight grid (w at the edge's
  sub-position, 0 elsewhere) selects + weights rows during the DVE
  multiply; a chunk-sum reduce completes the segment sum.
- dma_gather calls (1024 idxs each) round-robin over 4 SWDGE queues so Q7
  descriptor generation runs 4-wide.
- XW1 / HW2 tables are exchanged with 4 piece-wise AllGather collectives
  (piece-core-major table layout) overlapped with phase-1 / layer-1 compute.
"""

import sys

sys.path.insert(0, "/opt/trn_rl_repo")

import numpy as np

N = 50000
E = 800000
F = 512
HID = 64
NCLS = 16
NCORES = 8
P = 128
NPC = N // NCORES  # 6250
TILES = 49
TROWS = TILES * P  # 6272
CAP = 40  # max chunks per gather stage
NQ = 4  # SWDGE queues
# AllGather pieces: tile groups whose collectives overlap phase-1/layer-1
PIECE_T = [(0, 13), (13, 25), (25, 37), (37, 49)]
PIECE_ROWS = [(t1 - t0) * P for (t0, t1) in PIECE_T]   # rows per core per piece
CORE_BASE = [t0 * P for (t0, t1) in PIECE_T]           # row offset within a core
PIECE_BASE = [0]                                       # row offset in full table
for _pr in PIECE_ROWS[:-1]:
    PIECE_BASE.append(PIECE_BASE[-1] + NCORES * _pr)
# issue each piece's collective a few tiles after its data is complete so the
# Pool-queue SEQ wait (which blocks later gathers) finds the writes done
AG1_AT = [15, 27, 39, TILES - 1]
AG2_AT = [18, 30, 42, TILES - 1]

_TRACE = False
LAST_EXEC_NS = None


def _build_grid(es_row, dl, ew, shift):
    """Slot grid for one core+layer: tokens = row>>shift, dedup (dst, token).

    Returns ig [P, ctot] int16 token grid, wsub [P, ctot*S] f32 sub-weights,
    cnt[TILES] per-tile chunk counts, stages list, off[TILES] col offsets.
    """
    S = 1 << shift
    tok = es_row >> shift
    sub = es_row & (S - 1)
    ntok_max = int(tok.max()) + 1 if len(tok) else 1
    order = np.lexsort((tok, dl))
    dls, toks, subs, ews = dl[order], tok[order], sub[order], ew[order]
    key = dls * ntok_max + toks
    new = np.r_[True, np.diff(key) != 0]
    slot_id = np.cumsum(new) - 1  # per (sorted) edge -> slot
    slot_dl = dls[new]
    slot_tok = toks[new]
    nslots = len(slot_tok)

    kd = np.bincount(slot_dl, minlength=NPC)  # unique-token count per node
    kd_pad = np.concatenate([kd, np.zeros(TROWS - NPC, np.int64)])
    cnt = kd_pad.reshape(TILES, P).max(1)  # per-tile chunk count

    # stages: consecutive tiles while chunk sum <= CAP
    stages = []
    off = np.zeros(TILES, dtype=np.int64)
    t0 = 0
    ctot = 0
    while t0 < TILES:
        t1, cs = t0, 0
        while t1 < TILES and cs + cnt[t1] <= CAP:
            off[t1] = ctot + cs
            cs += cnt[t1]
            t1 += 1
        stages.append((t0, t1, int(cs), ctot))
        ctot += cs
        t0 = t1
    ctot = int(ctot)

    # slot columns: per node, j-th slot at off[tile] + j
    starts = np.r_[0, np.cumsum(kd)[:-1]]
    j = np.arange(nslots) - starts[slot_dl]
    tl = slot_dl // P
    prow = slot_dl % P
    col = off[tl] + j

    ig = np.zeros((P, ctot), dtype=np.int16)
    ig[prow, col] = slot_tok.astype(np.int16)
    wsub = np.zeros((P, ctot * S), dtype=np.float32)
    e_col = col[slot_id]  # per sorted edge
    e_prow = prow[slot_id]
    np.add.at(wsub, (e_prow, e_col * S + subs), ews)
    return ig, wsub, cnt, stages, off, ctot


def _preprocess(src, dst, edge_weight):
    src = np.asarray(src).astype(np.int64).ravel()
    dst = np.asarray(dst).astype(np.int64).ravel()
    w = np.asarray(edge_weight).astype(np.float32).ravel()

    tdeg = np.bincount(dst, minlength=N)
    grank = np.empty(N, dtype=np.int64)
    grank[np.argsort(-tdeg, kind="stable")] = np.arange(N)
    owner_of = grank % NCORES
    lid_of = grank // NCORES  # descending-degree local order
    # full tables are piece-major then core-major (AllGather piece layout)
    piece_of = np.searchsorted(np.array(CORE_BASE + [TROWS]), lid_of, side="right") - 1
    pb = np.array(PIECE_BASE)[piece_of]
    prw = np.array(PIECE_ROWS)[piece_of]
    cb = np.array(CORE_BASE)[piece_of]
    row_of = pb + owner_of * prw + (lid_of - cb)  # global table row

    owner_dst = owner_of[dst]
    cores = []
    for r in range(NCORES):
        m = owner_dst == r
        es, ed, ew = src[m], dst[m], w[m]
        dl = lid_of[ed]
        g1 = _build_grid(row_of[es], dl, ew, 1)  # layer 1: pair tokens
        g2 = _build_grid(row_of[es], dl, ew, 2)  # layer 2: 4-row fp32 tokens
        cores.append((g1, g2))

    # unify stage/offset structure across cores (max per-tile counts) so one
    # compiled program fits all cores
    def unify(idx):
        cnt = np.stack([c[idx][2] for c in cores]).max(0)
        stages = []
        off = np.zeros(TILES, dtype=np.int64)
        t0 = 0
        ctot = 0
        while t0 < TILES:
            t1, cs = t0, 0
            while t1 < TILES and cs + cnt[t1] <= CAP:
                off[t1] = ctot + cs
                cs += cnt[t1]
                t1 += 1
            stages.append((t0, t1, int(cs), ctot))
            ctot += cs
            t0 = t1
        return cnt, stages, off, int(ctot)

    uni1 = unify(0)
    uni2 = unify(1)

    # re-grid each core onto the unified layout
    def regrid(r, idx, uni, shift):
        S = 1 << shift
        ig_c, wsub_c, cnt_c, stages_c, off_c, ctot_c = cores[r][idx]
        cnt_u, stages_u, off_u, ctot_u = uni
        ig = np.zeros((P, ctot_u), dtype=np.int16)
        wsub = np.zeros((P, ctot_u * S), dtype=np.float32)
        for t in range(TILES):
            c = int(cnt_c[t])
            if c == 0:
                continue
            src_lo = int(off_c[t])
            dst_lo = int(off_u[t])
            ig[:, dst_lo:dst_lo + c] = ig_c[:, src_lo:src_lo + c]
            wsub[:, dst_lo * S:(dst_lo + c) * S] = wsub_c[:, src_lo * S:(src_lo + c) * S]
        return ig, wsub

    grids = []
    for r in range(NCORES):
        ig1, ws1 = regrid(r, 0, uni1, 1)
        ig2, ws2 = regrid(r, 1, uni2, 2)
        grids.append((ig1, ws1, ig2, ws2))

    layout = dict(uni1=uni1, uni2=uni2, owner=owner_of, lid=lid_of)
    return layout, grids


def _wrap_idx(ig):
    """[128, C] token grid -> dma_gather wrapped idx array [128, C*8] int16."""
    seq = ig.T.reshape(-1)  # position q = c*128 + p
    cols = seq.shape[0] // 16
    seqm = seq.reshape(cols, 16).T  # [16, cols]
    return np.tile(seqm, (8, 1)).astype(np.int16)  # [128, cols]


def _build(layout):
    import concourse.bacc as bacc
    import concourse.tile as tile
    import concourse.mybir as mybir
    from concourse.masks import make_identity

    cnt1, stages1, off1, ctot1 = layout["uni1"]
    cnt2, stages2, off2, ctot2 = layout["uni2"]
    fp32 = mybir.dt.float32

    nc = bacc.Bacc(
        "TRN2", target_bir_lowering=False, debug=False, num_devices=NCORES,
        num_swdge_queues=NQ,
    )
    x_in = nc.dram_tensor("x", [P, TILES * F], fp32, kind="ExternalInput")  # partition-major x^T blocks
    w1_in = nc.dram_tensor("w1", [F, HID], fp32, kind="ExternalInput")
    w2_in = nc.dram_tensor("w2", [HID, NCLS], fp32, kind="ExternalInput")
    b1_in = nc.dram_tensor("b1r", [P, HID], fp32, kind="ExternalInput")
    b2_in = nc.dram_tensor("b2r", [P, NCLS], fp32, kind="ExternalInput")
    idx1_in = nc.dram_tensor("idx1", [P, ctot1 * 8], mybir.dt.int16, kind="ExternalInput")
    wg1_in = nc.dram_tensor("wg1", [P, ctot1 * 2], fp32, kind="ExternalInput")
    idx2_in = nc.dram_tensor("idx2", [P, ctot2 * 8], mybir.dt.int16, kind="ExternalInput")
    wg2_in = nc.dram_tensor("wg2", [P, ctot2 * 4], fp32, kind="ExternalInput")
    out_d = nc.dram_tensor("out", [TROWS, NCLS], fp32, kind="ExternalOutput")

    xw1_shard = nc.dram_tensor("xw1_shard", [TROWS, HID], fp32)
    xw1_full = nc.dram_tensor("xw1_full", [NCORES * TROWS, HID], fp32, addr_space="Shared")
    hw2_shard = nc.dram_tensor("hw2_shard", [TROWS, NCLS], fp32)
    hw2_full = nc.dram_tensor("hw2_full", [NCORES * TROWS, NCLS], fp32, addr_space="Shared")

    rg = [list(range(NCORES))]
    qctr = [0]

    with tile.TileContext(nc) as tc:
        with (
            tc.tile_pool(name="const", bufs=1) as cpool,
            tc.tile_pool(name="xp", bufs=3) as xp,
            tc.tile_pool(name="xtp", bufs=3) as xtp,
            tc.tile_pool(name="gp", bufs=3) as gp,
            tc.tile_pool(name="gwp", bufs=2) as gwp,
            tc.tile_pool(name="hp", bufs=3) as hp,
            tc.tile_pool(name="ps", bufs=2, space="PSUM") as ps,
            tc.tile_pool(name="ps2", bufs=2, space="PSUM") as ps2,
        ):
            ident = cpool.tile([P, P], fp32)
            make_identity(nc, ident[:])
            w1t = cpool.tile([P, F // P, HID], fp32)  # [128, 4, 64] K-chunks
            nc.sync.dma_start(out=w1t[:], in_=w1_in[:].rearrange("(c p) h -> p c h", p=P))
            w2t = cpool.tile([HID, NCLS], fp32)
            nc.sync.dma_start(out=w2t[:], in_=w2_in[:])
            b1t = cpool.tile([P, HID], fp32)
            nc.sync.dma_start(out=b1t[:], in_=b1_in[:])
            b2t = cpool.tile([P, NCLS], fp32)
            nc.sync.dma_start(out=b2t[:], in_=b2_in[:])
            # ---- Phase 1: XW1 = x @ W1 per row-tile (x arrives transposed) ----
            for t in range(TILES):
                mm = ps2.tile([P, HID], fp32, space="PSUM", tag="mm1")
                xts = xtp.tile([P, F // P, P], fp32, tag="xts")
                nc.sync.dma_start(
                    out=xts[:],
                    in_=x_in[:, t * F : (t + 1) * F].rearrange("p (c j) -> p c j", j=P),
                )
                for c in range(F // P):
                    nc.tensor.matmul(
                        out=mm[:], lhsT=xts[:, c, :], rhs=w1t[:, c, :],
                        start=(c == 0), stop=(c == F // P - 1),
                    )
                xw1_sb = xp.tile([P, HID], fp32, tag="xw1sb")
                nc.any.tensor_copy(xw1_sb[:], mm[:])
                nc.sync.dma_start(out=xw1_shard[t * P : (t + 1) * P, :], in_=xw1_sb[:])
                for pi, at in enumerate(AG1_AT):
                    if t == at:
                        cb, pr, pb = CORE_BASE[pi], PIECE_ROWS[pi], PIECE_BASE[pi]
                        nc.gpsimd.collective_compute(
                            "AllGather", mybir.AluOpType.bypass, replica_groups=rg,
                            ins=[xw1_shard[cb : cb + pr, :]],
                            outs=[xw1_full[pb : pb + NCORES * pr, :]],
                        )

            # slot tables load during phase 1 / AllGather (needed from phase 3)
            idx1t = cpool.tile([P, ctot1 * 8], mybir.dt.int16)
            nc.sync.dma_start(out=idx1t[:], in_=idx1_in[:])
            wg1t = cpool.tile([P, ctot1 * 2], fp32)
            nc.sync.dma_start(out=wg1t[:], in_=wg1_in[:])
            idx2t = cpool.tile([P, ctot2 * 8], mybir.dt.int16)
            nc.sync.dma_start(out=idx2t[:], in_=idx2_in[:])
            wg2t = cpool.tile([P, ctot2 * 4], fp32)
            nc.sync.dma_start(out=wg2t[:], in_=wg2_in[:])

            # ---- Phases 3/5: aggregation layers ----
            def agg_layer(table_ap, idxt, wgt, S, width, cnt, stages, off, out_fn, gdt, esz):
                # products in fp32 regardless of table dtype (softmax precision)
                for (t0, t1, cs, c0) in stages:
                    g = gp.tile([P, CAP, esz], gdt, tag="g")
                    # 8 chunks/call: the gather ucode caps at 1024 idxs/call
                    for o in range(0, cs, 8):
                        n = min(8, cs - o)
                        nc.gpsimd.dma_gather(
                            out_ap=g[:, o : o + n, :], in_ap=table_ap,
                            idxs_ap=idxt[:, (c0 + o) * 8 : (c0 + o + n) * 8],
                            num_idxs=n * P, num_idxs_reg=n * P,
                            elem_size=esz, single_packet=True,
                            queue_num=qctr[0] % NQ,
                        )
                        qctr[0] += 1
                    gv = g[:].rearrange("p c (s f) -> p (c s) f", s=S)
                    gw = gwp.tile([P, CAP * S, width], fp32, tag=f"gw{S}")
                    nc.vector.tensor_tensor(
                        out=gw[:, 0 : cs * S, :],
                        in0=gv[:, 0 : cs * S, :],
                        in1=wgt[:, c0 * S : (c0 + cs) * S].to_broadcast(
                            [P, cs * S, width]
                        ),
                        op=mybir.AluOpType.mult,
                    )
                    for t in range(t0, t1):
                        c = int(cnt[t])
                        if c == 0:
                            continue
                        lo = int(off[t]) - c0
                        red = hp.tile([P, width], fp32, tag=f"red{width}")
                        nc.vector.tensor_reduce(
                            out=red[:], in_=gw[:, lo * S : (lo + c) * S, :].rearrange("p c d -> p d c"),
                            axis=mybir.AxisListType.X, op=mybir.AluOpType.add,
                        )
                        out_fn(t, red)

            # Layer 1 epilogue per tile: h=relu(agg+b1); hw2 = h@W2
            def l1_out(t, red):
                h = hp.tile([P, HID], fp32, tag="h")
                nc.vector.tensor_tensor(out=h[:], in0=red[:], in1=b1t[:], op=mybir.AluOpType.add)
                nc.scalar.activation(h[:], h[:], mybir.ActivationFunctionType.Relu)
                ht_ps = ps.tile([P, P], fp32, space="PSUM", tag="tp")
                nc.tensor.transpose(out=ht_ps[0:HID, :], in_=h[:], identity=ident[:])
                ht = xtp.tile([HID, P], fp32, tag="ht")
                nc.any.tensor_copy(ht[:], ht_ps[0:HID, :])
                mm2 = ps2.tile([P, NCLS], fp32, space="PSUM", tag="mm2")
                nc.tensor.matmul(out=mm2[:], lhsT=ht[:], rhs=w2t[:], start=True, stop=True)
                hw2 = hp.tile([P, NCLS], fp32, tag="hw2")
                nc.any.tensor_copy(hw2[:], mm2[:])
                nc.sync.dma_start(out=hw2_shard[t * P : (t + 1) * P, :], in_=hw2[:])
                for pi, at in enumerate(AG2_AT):
                    if t == at:
                        cb, pr, pb = CORE_BASE[pi], PIECE_ROWS[pi], PIECE_BASE[pi]
                        nc.gpsimd.collective_compute(
                            "AllGather", mybir.AluOpType.bypass, replica_groups=rg,
                            ins=[hw2_shard[cb : cb + pr, :]],
                            outs=[hw2_full[pb : pb + NCORES * pr, :]],
                        )

            table1 = xw1_full[:].rearrange("(t s) f -> t (s f)", s=2)
            agg_layer(table1, idx1t, wg1t, 2, HID, cnt1, stages1, off1, l1_out, fp32, P)

            # ---- Phase 5: layer 2 + batched softmax ----
            logits = cpool.tile([P, TILES, NCLS], fp32)

            def l2_out(t, red):
                nc.vector.tensor_tensor(
                    out=logits[:, t, :], in0=red[:], in1=b2t[:], op=mybir.AluOpType.add
                )

            table2 = hw2_full[:].rearrange("(t s) f -> t (s f)", s=4)
            agg_layer(table2, idx2t, wg2t, 4, NCLS, cnt2, stages2, off2, l2_out, fp32, HID)

            mx = cpool.tile([P, TILES], fp32)
            nc.vector.tensor_reduce(out=mx[:], in_=logits[:], axis=mybir.AxisListType.X, op=mybir.AluOpType.max)
            sh = cpool.tile([P, TILES, NCLS], fp32)
            nc.vector.tensor_tensor(
                out=sh[:], in0=logits[:],
                in1=mx[:].to_broadcast([P, TILES, NCLS]),
                op=mybir.AluOpType.subtract,
            )
            nc.scalar.activation(sh[:], sh[:], mybir.ActivationFunctionType.Exp)
            sm = cpool.tile([P, TILES], fp32)
            nc.vector.tensor_reduce(out=sm[:], in_=sh[:], axis=mybir.AxisListType.X, op=mybir.AluOpType.add)
            nc.vector.reciprocal(sm[:], sm[:])
            nc.vector.tensor_tensor(
                out=sh[:], in0=sh[:],
                in1=sm[:].to_broadcast([P, TILES, NCLS]),
                op=mybir.AluOpType.mult,
            )
            nc.sync.dma_start(
                out=out_d[:].rearrange("(t p) c -> p t c", p=P), in_=sh[:]
            )
    nc.compile()
    return nc


def _prepare(x, src, dst, edge_weight, W1, b1, W2, b2):
    """Build the compiled program + per-core input maps + layout."""
    x = np.asarray(x, dtype=np.float32)
    W1 = np.asarray(W1, dtype=np.float32)
    b1 = np.asarray(b1, dtype=np.float32)
    W2 = np.asarray(W2, dtype=np.float32)
    b2 = np.asarray(b2, dtype=np.float32)

    layout, grids = _preprocess(src, dst, edge_weight)
    owner, lid = layout["owner"], layout["lid"]

    nc = _build(layout)

    b1r = np.broadcast_to(b1, (P, HID)).copy()
    b2r = np.broadcast_to(b2, (P, NCLS)).copy()
    in_maps = []
    for r in range(NCORES):
        xr = np.zeros((TROWS, F), dtype=np.float32)
        gl = np.flatnonzero(owner == r)
        xr[lid[gl]] = x[gl]
        # [P, TILES, C, P]: partition-major so each tile DMA reads 2KB/partition
        xr = np.ascontiguousarray(
            xr.reshape(TILES, P, F // P, P).transpose(3, 0, 2, 1)
        ).reshape(P, TILES * F)
        ig1, ws1, ig2, ws2 = grids[r]
        in_maps.append(
            {
                "x": xr, "w1": W1, "w2": W2, "b1r": b1r, "b2r": b2r,
                "idx1": _wrap_idx(ig1), "wg1": ws1,
                "idx2": _wrap_idx(ig2), "wg2": ws2,
            }
        )
    return nc, in_maps, layout


def _unshard(shards, layout):
    owner, lid = layout["owner"], layout["lid"]
    out = np.empty((N, NCLS), dtype=np.float32)
    for r in range(NCORES):
        gl = np.flatnonzero(owner == r)
        out[gl] = shards[r][lid[gl]]
    return out


def kernel(x, src, dst, edge_weight, W1, b1, W2, b2):
    global LAST_EXEC_NS
    from concourse import bass_utils

    nc, in_maps, layout = _prepare(x, src, dst, edge_weight, W1, b1, W2, b2)
    res = bass_utils.run_bass_kernel_spmd(
        nc, in_maps, core_ids=list(range(NCORES)), trace=_TRACE
    )
    LAST_EXEC_NS = res.exec_time_ns
    return _unshard([res.results[r]["out"] for r in range(NCORES)], layout)
